# revision 4
# baseline (speedup 1.0000x reference)
"""Causal GQA self-attention on 8 Trainium2 NeuronCores (bf16 pipeline).

Sharding: data-parallel over batch (4) x tensor-parallel over heads (2 halves
of 14 heads each, KV heads replicated for the shared GQA group). Each core
computes a partial output through the row-parallel out-projection; the host
sums the two partials per batch element.

Per-core local structure (local head h = 4*g + j, 4 kv groups, group 3 has
2 heads):
  half 0: global heads [0..11, 24, 25],  kv heads [0, 1, 2, 6]
  half 1: global heads [12..23, 26, 27], kv heads [3, 4, 5, 6]

Layouts (SBUF, all bf16 unless noted):
  xT  [128, 7, 2048]  x^T (C on partitions), host-pretransposed, DMA'd in
  QT  [128, 4, 2048]  Q^T: head (g, j) at partitions 32g:32g+32, chunk j
  KT  [128, 2048]     K^T: group g at partitions 32g:32g+32 (single copy)
  V   [128, 16, 4, 64] kpos on partitions; cols 0:32 data, 32:64 ones
  AOT [128, 4, 2048]  attention out^T: head h at (partitions 32*(h%4), h//4)

Per (q-block, head-pair): S^T = K^T-stationary x Q^T-moving row-banded
matmuls (tile_position=(32g, 0)); exp on ScalarE (scale folded); causal mask
multiply on DVE; P^T feeds column-tiled AV matmuls where V's ones-columns
produce the softmax denominators in the same matmul; a selection matmul
aligns denominators with data partitions; reciprocal+multiply on DVE.
Projection and out-projection matmuls are interleaved between attention
pairs to keep PE saturated.
"""

import sys

sys.path.insert(0, "/opt/trn_rl_repo")

from contextlib import ExitStack

import numpy as np
import ml_dtypes

import concourse.bass as bass
import concourse.mybir as mybir
import concourse.tile as tile
from concourse import bacc
from concourse.bass import ts
from concourse.bass_utils import run_bass_kernel_spmd

F32 = mybir.dt.float32
BF16 = mybir.dt.bfloat16
EXP = mybir.ActivationFunctionType.Exp
P = 128
T, C = 2048, 896
D = 32
HL = 14  # local heads per core
GL = 4  # local kv groups per core
DH = HL * D  # 448
SCALE = 1.0 / float(np.sqrt(D))

HEADS_HALF = [
    list(range(0, 12)) + [24, 25],
    list(range(12, 24)) + [26, 27],
]
KV_HALF = [[0, 1, 2, 6], [3, 4, 5, 6]]

# head pairs per group: (g, j0) covers local heads 4g+j0, 4g+j0+1
PAIRS = [(g, j0) for g in range(4) for j0 in (0, 2) if not (g == 3 and j0 == 2)]


def _trace(tc, d):
    nc = tc.nc
    with ExitStack() as ctx:
        const = ctx.enter_context(tc.tile_pool(name="const", bufs=1))
        maskb = const.tile([P, P], BF16)
        nc.sync.dma_start(maskb[:], d["mask"][:])
        sel = const.tile([P, P], BF16)
        nc.sync.dma_start(sel[:], d["sel"][:])

        persist = ctx.enter_context(tc.tile_pool(name="persist", bufs=1))
        xT = persist.tile([P, 7, T], BF16, tag="xT")
        QT = persist.tile([P, 4, T], BF16, tag="QT")
        KT = persist.tile([P, T], BF16, tag="KT")
        V = persist.tile([P, 16, GL, 64], BF16, tag="V")
        AOT = persist.tile([P, 4, T], BF16, tag="AOT")
        wq = persist.tile([P, 7, 512], BF16, tag="wq")
        nc.sync.dma_start(wq[:], d["wq"].rearrange("p (c n) -> p c n", c=7))
        wk = persist.tile([P, 7, P], BF16, tag="wk")
        nc.sync.dma_start(wk[:], d["wk"].rearrange("p (c n) -> p c n", c=7))
        wv = persist.tile([P, 7, P], BF16, tag="wv")
        nc.sync.dma_start(wv[:], d["wv"].rearrange("p (c n) -> p c n", c=7))
        wo = persist.tile([P, 4, C], BF16, tag="wo")
        nc.sync.dma_start(wo[:], d["wo"].rearrange("p (c n) -> p c n", c=4))

        nc.gpsimd.memset(V[:, :, :, D:64], 1.0)

        xtv = d["xt"].rearrange("(co ci) t -> ci co t", ci=P)
        for nk in range(4):
            nc.sync.dma_start(xT[:, :, ts(nk, 512)], xtv[:, :, ts(nk, 512)])
        ov = d["out"].rearrange("(to ti) c -> ti to c", ti=P)

        sp_pool = ctx.enter_context(
            tc.tile_pool(name="sp", bufs=2, space="PSUM")
        )
        av_pool = ctx.enter_context(
            tc.tile_pool(name="avp", bufs=2, space="PSUM")
        )
        px_pool = ctx.enter_context(
            tc.tile_pool(name="px", bufs=2, space="PSUM")
        )
        pt_pool = ctx.enter_context(tc.tile_pool(name="pt", bufs=2))
        avs_pool = ctx.enter_context(tc.tile_pool(name="avs", bufs=2))
        zr_pool = ctx.enter_context(tc.tile_pool(name="zr", bufs=2))
        ao_pool = ctx.enter_context(tc.tile_pool(name="ao", bufs=2))
        ob_pool = ctx.enter_context(tc.tile_pool(name="ob", bufs=2))

        # ---------------- PE filler units ----------------
        def q_unit(nk, mc):
            def emit():
                ps = px_pool.tile([P, 512], F32, tag="px")
                for c in range(7):
                    nc.tensor.matmul(
                        ps[:],
                        lhsT=wq[:, c, ts(mc, P)],
                        rhs=xT[:, c, ts(nk, 512)],
                        start=(c == 0),
                        stop=(c == 6),
                    )
                nc.vector.tensor_copy(QT[:, mc, ts(nk, 512)], ps[:])

            return emit

        def k_unit(nk):
            def emit():
                ps = px_pool.tile([P, 512], F32, tag="px")
                for c in range(7):
                    nc.tensor.matmul(
                        ps[:],
                        lhsT=wk[:, c, :],
                        rhs=xT[:, c, ts(nk, 512)],
                        start=(c == 0),
                        stop=(c == 6),
                    )
                nc.vector.tensor_copy(KT[:, ts(nk, 512)], ps[:])

            return emit

        def v_unit(tcq):
            def emit():
                ps = px_pool.tile([P, 512], F32, tag="px")
                for c in range(7):
                    nc.tensor.matmul(
                        ps[:, 0:P],
                        lhsT=xT[:, c, ts(tcq, P)],
                        rhs=wv[:, c, :],
                        start=(c == 0),
                        stop=(c == 6),
                    )
                nc.vector.tensor_copy(
                    V[:, tcq, :, 0:D],
                    ps[:, 0:P].rearrange("p (g e) -> p g e", g=GL),
                )

            return emit

        def o_unit(qc, tcl):
            def emit():
                tg = qc * 4 + tcl
                t0 = tg * P
                ob = ob_pool.tile([P, C], F32, tag="ob")
                for ncol in range(2):
                    po = px_pool.tile([P, 512], F32, tag="px")
                    for mc in range(4):
                        K = P if mc < 3 else 64
                        nc.tensor.matmul(
                            po[:, 0:448],
                            lhsT=AOT[0:K, mc, t0 : t0 + P],
                            rhs=wo[0:K, mc, ncol * 448 : (ncol + 1) * 448],
                            start=(mc == 0),
                            stop=(mc == 3),
                        )
                    nc.vector.tensor_copy(
                        ob[:, ncol * 448 : (ncol + 1) * 448], po[:, 0:448]
                    )
                nc.sync.dma_start(ov[:, tg, :], ob[:])

            return emit

        def proj_units(nk):
            return (
                [k_unit(nk)]
                + [q_unit(nk, mc) for mc in range(4)]
                + [v_unit(4 * nk + i) for i in range(4)]
            )

        # ---------------- attention ----------------
        def attention_pair(qc, g, j0, fillers):
            qs = qc * 512
            nks = 4 * (qc + 1)
            av = av_pool.tile([P, 512], F32, tag="av")

            def av_mm(ki, pt_t, qoff):
                for j2 in range(2):
                    nc.tensor.matmul(
                        av[ts(j2, 64), qoff:512],
                        lhsT=V[:, ki, g, 0:64],
                        rhs=pt_t[:, j2, qoff:512],
                        start=(ki == 0),
                        stop=(ki == nks - 1),
                        skip_group_check=True,
                        tile_position=(0, 64 * j2),
                    )

            prev = None
            for ki in range(nks):
                ks = ki * P
                qoff = max(0, ks - qs)
                sp_t = sp_pool.tile([P, 2, 512], F32, tag="sp")
                for j2 in range(2):
                    j = j0 + j2
                    nc.tensor.matmul(
                        sp_t[:, j2, qoff:512],
                        lhsT=KT[ts(g, D), ks : ks + P],
                        rhs=QT[ts(g, D), j, qs + qoff : qs + 512],
                        start=True,
                        stop=True,
                        tile_position=(g * D, 0),
                    )
                pt_t = pt_pool.tile([P, 2, 512], BF16, tag="pt")
                nc.scalar.activation(
                    pt_t[:, :, qoff:512], sp_t[:, :, qoff:512], EXP, scale=SCALE
                )
                if ks >= qs:  # diagonal chunk: zero the upper triangle
                    nc.vector.tensor_tensor(
                        pt_t[:, :, qoff : qoff + P],
                        pt_t[:, :, qoff : qoff + P],
                        maskb[:, None, :].to_broadcast((P, 2, P)),
                        mybir.AluOpType.mult,
                    )
                if prev is not None:
                    av_mm(*prev)
                prev = (ki, pt_t, qoff)
            av_mm(*prev)

            # softmax epilogue: move denominators onto data partitions
            avs = avs_pool.tile([P, 512], BF16, tag="avs")
            nc.vector.tensor_copy(avs[:], av[:])
            zq = px_pool.tile([P, 512], F32, tag="px")
            nc.tensor.matmul(zq[:], lhsT=sel[:], rhs=avs[:], start=True, stop=True)
            zr = zr_pool.tile([P, 512], F32, tag="zr")
            nc.vector.reciprocal_approx_fast(zr[:], zq[:])
            ao = ao_pool.tile([P, 512], BF16, tag="ao")
            nc.vector.tensor_tensor(ao[:], avs[:], zr[:], mybir.AluOpType.mult)
            nc.sync.dma_start(
                AOT[ts(j0, D), g, qs : qs + 512], ao[0:D, :]
            )
            nc.sync.dma_start(
                AOT[ts(j0 + 1, D), g, qs : qs + 512], ao[64 : 64 + D, :]
            )

        # ---------------- main schedule ----------------
        for u in proj_units(0):
            u()
        for qc in range(4):
            fillers = []
            if qc < 3:
                fillers += proj_units(qc + 1)
            if qc > 0:
                fillers += [o_unit(qc - 1, tcl) for tcl in range(4)]
            fi = 0
            for pi, (g, j0) in enumerate(PAIRS):
                attention_pair(qc, g, j0, fillers)
                # spread remaining fillers over remaining pair slots
                want = ((pi + 1) * len(fillers)) // len(PAIRS)
                while fi < want:
                    fillers[fi]()
                    fi += 1
            while fi < len(fillers):
                fillers[fi]()
                fi += 1
        for tcl in range(4):
            o_unit(3, tcl)()


_NC_CACHE = None


def _build():
    global _NC_CACHE
    if _NC_CACHE is not None:
        return _NC_CACHE
    nc = bacc.Bacc("TRN2", target_bir_lowering=False, debug=False, num_devices=8)
    d = {
        "xt": nc.dram_tensor("xt", (C, T), BF16, kind="ExternalInput"),
        "wq": nc.dram_tensor("wq", (P, 7 * 512), BF16, kind="ExternalInput"),
        "wk": nc.dram_tensor("wk", (P, 7 * P), BF16, kind="ExternalInput"),
        "wv": nc.dram_tensor("wv", (P, 7 * P), BF16, kind="ExternalInput"),
        "wo": nc.dram_tensor("wo", (P, 4 * C), BF16, kind="ExternalInput"),
        "mask": nc.dram_tensor("mask", (P, P), BF16, kind="ExternalInput"),
        "sel": nc.dram_tensor("sel", (P, P), BF16, kind="ExternalInput"),
        "out": nc.dram_tensor("out", (T, C), F32, kind="ExternalOutput"),
    }
    with tile.TileContext(nc) as tc:
        _trace(tc, {k: v[:] for k, v in d.items()})
    nc.compile()
    _NC_CACHE = nc
    return nc


def _to_bf16(a):
    return np.ascontiguousarray(a.astype(ml_dtypes.bfloat16))


def _row_blocks(a, nblk):
    """[nblk*128, n] f32 -> [128, nblk, n] (row r = 128*co + ci -> [ci, co])."""
    n = a.shape[1]
    return a.reshape(nblk, P, n).transpose(1, 0, 2).reshape(P, nblk * n)


def _in_maps(x, Wq, Wk, Wv, Wo):
    maskb = (np.arange(P)[None, :] >= np.arange(P)[:, None]).astype(np.float32)
    selm = np.zeros((P, P), dtype=np.float32)
    for m in range(P):
        k = m if (m // D) % 2 == 1 else m + D
        selm[k, m] = 1.0
    maps = []
    for c in range(8):
        b, hf = c // 2, c % 2
        hcols = np.concatenate(
            [np.arange(D * h, D * h + D) for h in HEADS_HALF[hf]]
        )
        kcols = np.concatenate(
            [np.arange(D * g, D * g + D) for g in KV_HALF[hf]]
        )
        # Q columns: local (g-major) -> [j, g, d] blocks of 128, zero-padded
        wq_full = Wq[:, hcols]  # [896, 448], head-major h = 4g+j
        wq_pad = np.zeros((C, 512), dtype=np.float32)
        wq_pad[:, :DH] = wq_full
        wq_p = (
            wq_pad.reshape(C, 4, 4, D).transpose(0, 2, 1, 3).reshape(C, 512)
        )
        wo_full = Wo[hcols, :]  # [448, 896]
        wo_pad = np.zeros((512, C), dtype=np.float32)
        wo_pad[:DH] = wo_full
        maps.append(
            {
                "xt": _to_bf16(x[b].T),
                "wq": _to_bf16(_row_blocks(wq_p, 7)),
                "wk": _to_bf16(_row_blocks(Wk[:, kcols], 7)),
                "wv": _to_bf16(_row_blocks(Wv[:, kcols], 7)),
                "wo": _to_bf16(_row_blocks(wo_pad, 4)),
                "mask": _to_bf16(maskb),
                "sel": _to_bf16(selm),
            }
        )
    return maps


def run(x, Wq, Wk, Wv, Wo, trace=False):
    nc = _build()
    res = run_bass_kernel_spmd(
        nc, _in_maps(x, Wq, Wk, Wv, Wo), core_ids=list(range(8)), trace=trace
    )
    outs = [r["out"] for r in res.results]
    final = np.empty((4, T, C), np.float32)
    for b in range(4):
        final[b] = outs[2 * b] + outs[2 * b + 1]
    return final, res


def kernel(x, Wq, Wk, Wv, Wo):
    x = np.asarray(x, dtype=np.float32)
    out, _ = run(
        x,
        np.asarray(Wq, np.float32),
        np.asarray(Wk, np.float32),
        np.asarray(Wv, np.float32),
        np.asarray(Wo, np.float32),
    )
    return out


if __name__ == "__main__":
    pass


# revision 8
# speedup vs baseline: 1.1375x; 1.1375x over previous
"""Causal GQA self-attention on 8 Trainium2 NeuronCores (bf16 pipeline).

Sharding: data-parallel over batch (4) x tensor-parallel over heads (2 halves
of 14 heads each, KV heads replicated for the shared GQA group). Each core
computes a partial output through the row-parallel out-projection; the host
sums the two partials per batch element.

Per-core local structure (local head h = 4*g + j, 4 kv groups, group 3 has
2 heads):
  half 0: global heads [0..11, 24, 25],  kv heads [0, 1, 2, 6]
  half 1: global heads [12..23, 26, 27], kv heads [3, 4, 5, 6]

Layouts (SBUF, all bf16 unless noted):
  xT  [128, 7, 2048]  x^T (C on partitions), host-pretransposed, DMA'd in
  QT  [128, 4, 2048]  Q^T: head (g, j) at partitions 32g:32g+32, chunk j
  KT  [128, 2048]     K^T: group g at partitions 32g:32g+32 (single copy)
  V   [128, 16, 4, 64] kpos on partitions; cols 0:32 data, 32:64 ones
  AOT [128, 4, 2048]  attention out^T: head h at (partitions 32*(h%4), h//4)

Per (q-block, head-pair): S^T = K^T-stationary x Q^T-moving row-banded
matmuls (tile_position=(32g, 0)); exp on ScalarE (scale folded); causal mask
multiply on DVE; P^T feeds column-tiled AV matmuls where V's ones-columns
produce the softmax denominators in the same matmul; a selection matmul
aligns denominators with data partitions; reciprocal+multiply on DVE.
Projection and out-projection matmuls are interleaved between attention
pairs to keep PE saturated.
"""

import sys

sys.path.insert(0, "/opt/trn_rl_repo")

from contextlib import ExitStack

import numpy as np
import ml_dtypes

import concourse.bass as bass
import concourse.mybir as mybir
import concourse.tile as tile
from concourse import bacc
from concourse.bass import ts
from concourse.bass_utils import run_bass_kernel_spmd

F32 = mybir.dt.float32
BF16 = mybir.dt.bfloat16
EXP = mybir.ActivationFunctionType.Exp
P = 128
T, C = 2048, 896
D = 32
HL = 14  # local heads per core
GL = 4  # local kv groups per core
DH = HL * D  # 448
SCALE = 1.0 / float(np.sqrt(D))

HEADS_HALF = [
    list(range(0, 12)) + [24, 25],
    list(range(12, 24)) + [26, 27],
]
KV_HALF = [[0, 1, 2, 6], [3, 4, 5, 6]]

# head pairs per group: (g, j0) covers local heads 4g+j0, 4g+j0+1
PAIRS = [(g, j0) for g in range(4) for j0 in (0, 2) if not (g == 3 and j0 == 2)]


def _trace(tc, d):
    nc = tc.nc
    with ExitStack() as ctx:
        const = ctx.enter_context(tc.tile_pool(name="const", bufs=1))
        maskb = const.tile([P, P], BF16)
        sel = const.tile([P, P], BF16)

        persist = ctx.enter_context(tc.tile_pool(name="persist", bufs=1))
        xT = persist.tile([P, 7, T], BF16, tag="xT")
        QT = persist.tile([P, 4, T], BF16, tag="QT")
        KT = persist.tile([P, T], BF16, tag="KT")
        V = persist.tile([P, 16, GL, 64], BF16, tag="V")
        AOT = persist.tile([P, 4, T], BF16, tag="AOT")
        wq = persist.tile([P, 7, 512], BF16, tag="wq")
        wk = persist.tile([P, 7, P], BF16, tag="wk")
        wv = persist.tile([P, 7, P], BF16, tag="wv")
        wo = persist.tile([P, 4, C], BF16, tag="wo")

        # load order: what the first projection unit needs comes first
        xtv = d["xt"].rearrange("(co ci) t -> ci co t", ci=P)
        nc.sync.dma_start(wk[:], d["wk"].rearrange("p (c n) -> p c n", c=7))
        nc.sync.dma_start(xT[:, :, ts(0, 512)], xtv[:, :, ts(0, 512)])
        nc.sync.dma_start(wq[:], d["wq"].rearrange("p (c n) -> p c n", c=7))
        nc.sync.dma_start(wv[:], d["wv"].rearrange("p (c n) -> p c n", c=7))
        nc.sync.dma_start(maskb[:], d["mask"][:])
        nc.sync.dma_start(sel[:], d["sel"][:])
        nc.gpsimd.memset(V[:, :, :, D:64], 1.0)
        for nk in range(1, 4):
            nc.sync.dma_start(xT[:, :, ts(nk, 512)], xtv[:, :, ts(nk, 512)])
        nc.sync.dma_start(wo[:], d["wo"].rearrange("p (c n) -> p c n", c=4))
        ov = d["out"].rearrange("(to ti) c -> ti to c", ti=P)

        sp_pool = ctx.enter_context(
            tc.tile_pool(name="sp", bufs=2, space="PSUM")
        )
        av_pool = ctx.enter_context(
            tc.tile_pool(name="avp", bufs=2, space="PSUM")
        )
        px_pool = ctx.enter_context(
            tc.tile_pool(name="px", bufs=2, space="PSUM")
        )
        pt_pool = ctx.enter_context(tc.tile_pool(name="pt", bufs=2))
        avs_pool = ctx.enter_context(tc.tile_pool(name="avs", bufs=2))
        zr_pool = ctx.enter_context(tc.tile_pool(name="zr", bufs=2))
        ao_pool = ctx.enter_context(tc.tile_pool(name="ao", bufs=2))
        ob_pool = ctx.enter_context(tc.tile_pool(name="ob", bufs=2))

        # ---------------- PE filler units ----------------
        def q_unit(nk, mc):
            def emit():
                ps = px_pool.tile([P, 512], F32, tag="px")
                for c in range(7):
                    nc.tensor.matmul(
                        ps[:],
                        lhsT=wq[:, c, ts(mc, P)],
                        rhs=xT[:, c, ts(nk, 512)],
                        start=(c == 0),
                        stop=(c == 6),
                    )
                nc.vector.tensor_copy(QT[:, mc, ts(nk, 512)], ps[:])

            return emit

        def k_unit(nk):
            def emit():
                ps = px_pool.tile([P, 512], F32, tag="px")
                for c in range(7):
                    nc.tensor.matmul(
                        ps[:],
                        lhsT=wk[:, c, :],
                        rhs=xT[:, c, ts(nk, 512)],
                        start=(c == 0),
                        stop=(c == 6),
                    )
                nc.vector.tensor_copy(KT[:, ts(nk, 512)], ps[:])

            return emit

        def v_unit(tcq):
            def emit():
                ps = px_pool.tile([P, 512], F32, tag="px")
                for c in range(7):
                    nc.tensor.matmul(
                        ps[:, 0:P],
                        lhsT=xT[:, c, ts(tcq, P)],
                        rhs=wv[:, c, :],
                        start=(c == 0),
                        stop=(c == 6),
                    )
                nc.vector.tensor_copy(
                    V[:, tcq, :, 0:D],
                    ps[:, 0:P].rearrange("p (g e) -> p g e", g=GL),
                )

            return emit

        def o_unit(qc, tcl):
            def emit():
                tg = qc * 4 + tcl
                t0 = tg * P
                ob = ob_pool.tile([P, C], F32, tag="ob")
                for ncol in range(2):
                    po = px_pool.tile([P, 512], F32, tag="px")
                    for mc in range(4):
                        K = P if mc < 3 else 64
                        nc.tensor.matmul(
                            po[:, 0:448],
                            lhsT=AOT[0:K, mc, t0 : t0 + P],
                            rhs=wo[0:K, mc, ncol * 448 : (ncol + 1) * 448],
                            start=(mc == 0),
                            stop=(mc == 3),
                        )
                    nc.vector.tensor_copy(
                        ob[:, ncol * 448 : (ncol + 1) * 448], po[:, 0:448]
                    )
                nc.sync.dma_start(ov[:, tg, :], ob[:])

            return emit

        def proj_units(nk):
            return (
                [k_unit(nk)]
                + [q_unit(nk, mc) for mc in range(4)]
                + [v_unit(4 * nk + i) for i in range(4)]
            )

        # ---------------- attention ----------------
        def attention_pair(qc, g, j0, finish_prev):
            """Emit one pair's scores/exp/AV stream. The softmax epilogue's
            PE matmul is deferred: returns a `finish` closure the caller
            emits later (so the sel-matmul never blocks the PE queue on the
            DVE avs-copy)."""
            qs = qc * 512
            nks = 4 * (qc + 1)
            av = av_pool.tile([P, 512], F32, tag="av")

            def av_mm(ki, pt_t, qoff):
                for j2 in range(2):
                    nc.tensor.matmul(
                        av[ts(j2, 64), qoff:512],
                        lhsT=V[:, ki, g, 0:64],
                        rhs=pt_t[:, j2, qoff:512],
                        start=(ki == 0),
                        stop=(ki == nks - 1),
                        skip_group_check=True,
                        tile_position=(0, 64 * j2),
                    )

            prev = None
            for ki in range(nks):
                ks = ki * P
                qoff = max(0, ks - qs)
                sp_t = sp_pool.tile([P, 2, 512], F32, tag="sp")
                for j2 in range(2):
                    j = j0 + j2
                    nc.tensor.matmul(
                        sp_t[:, j2, qoff:512],
                        lhsT=KT[ts(g, D), ks : ks + P],
                        rhs=QT[ts(g, D), j, qs + qoff : qs + 512],
                        start=True,
                        stop=True,
                        tile_position=(g * D, 0),
                    )
                pt_t = pt_pool.tile([P, 2, 512], BF16, tag="pt")
                nc.scalar.activation(
                    pt_t[:, :, qoff:512], sp_t[:, :, qoff:512], EXP, scale=SCALE
                )
                if ks >= qs:  # diagonal chunk: zero the upper triangle
                    nc.vector.tensor_tensor(
                        pt_t[:, :, qoff : qoff + P],
                        pt_t[:, :, qoff : qoff + P],
                        maskb[:, None, :].to_broadcast((P, 2, P)),
                        mybir.AluOpType.mult,
                    )
                if prev is not None:
                    av_mm(*prev)
                prev = (ki, pt_t, qoff)
                if ki == 0 and finish_prev is not None:
                    finish_prev()
                    finish_prev = None
            av_mm(*prev)
            if finish_prev is not None:
                finish_prev()

            # softmax epilogue part 1: drain AV psum to SBUF
            avs = avs_pool.tile([P, 512], BF16, tag="avs")
            nc.vector.tensor_copy(avs[:], av[:])

            def finish():
                # move denominators onto data partitions, normalize, store
                zq = px_pool.tile([P, 512], F32, tag="px")
                nc.tensor.matmul(
                    zq[:], lhsT=sel[:], rhs=avs[:], start=True, stop=True
                )
                zr = zr_pool.tile([P, 512], F32, tag="zr")
                nc.vector.reciprocal_approx_fast(zr[:], zq[:])
                ao = ao_pool.tile([P, 512], BF16, tag="ao")
                nc.vector.tensor_tensor(
                    ao[:], avs[:], zr[:], mybir.AluOpType.mult
                )
                nc.sync.dma_start(AOT[ts(j0, D), g, qs : qs + 512], ao[0:D, :])
                nc.sync.dma_start(
                    AOT[ts(j0 + 1, D), g, qs : qs + 512], ao[64 : 64 + D, :]
                )

            return finish

        # ---------------- main schedule ----------------
        for u in proj_units(0):
            u()
        finish_prev = None
        for qc in range(4):
            fillers = []
            if qc < 3:
                fillers += proj_units(qc + 1)
            if qc > 0:
                fillers += [o_unit(qc - 1, tcl) for tcl in range(4)]
            fi = 0
            for pi, (g, j0) in enumerate(PAIRS):
                finish_prev = attention_pair(qc, g, j0, finish_prev)
                # spread remaining fillers over remaining pair slots
                want = ((pi + 1) * len(fillers)) // len(PAIRS)
                while fi < want:
                    fillers[fi]()
                    fi += 1
            while fi < len(fillers):
                fillers[fi]()
                fi += 1
        finish_prev()
        for tcl in range(4):
            o_unit(3, tcl)()


_NC_CACHE = None


def _build():
    global _NC_CACHE
    if _NC_CACHE is not None:
        return _NC_CACHE
    nc = bacc.Bacc("TRN2", target_bir_lowering=False, debug=False, num_devices=8)
    d = {
        "xt": nc.dram_tensor("xt", (C, T), BF16, kind="ExternalInput"),
        "wq": nc.dram_tensor("wq", (P, 7 * 512), BF16, kind="ExternalInput"),
        "wk": nc.dram_tensor("wk", (P, 7 * P), BF16, kind="ExternalInput"),
        "wv": nc.dram_tensor("wv", (P, 7 * P), BF16, kind="ExternalInput"),
        "wo": nc.dram_tensor("wo", (P, 4 * C), BF16, kind="ExternalInput"),
        "mask": nc.dram_tensor("mask", (P, P), BF16, kind="ExternalInput"),
        "sel": nc.dram_tensor("sel", (P, P), BF16, kind="ExternalInput"),
        "out": nc.dram_tensor("out", (T, C), F32, kind="ExternalOutput"),
    }
    with tile.TileContext(nc) as tc:
        _trace(tc, {k: v[:] for k, v in d.items()})
    nc.compile()
    _NC_CACHE = nc
    return nc


def _to_bf16(a):
    return np.ascontiguousarray(a.astype(ml_dtypes.bfloat16))


def _row_blocks(a, nblk):
    """[nblk*128, n] f32 -> [128, nblk, n] (row r = 128*co + ci -> [ci, co])."""
    n = a.shape[1]
    return a.reshape(nblk, P, n).transpose(1, 0, 2).reshape(P, nblk * n)


def _in_maps(x, Wq, Wk, Wv, Wo):
    maskb = (np.arange(P)[None, :] >= np.arange(P)[:, None]).astype(np.float32)
    selm = np.zeros((P, P), dtype=np.float32)
    for m in range(P):
        k = m if (m // D) % 2 == 1 else m + D
        selm[k, m] = 1.0
    maps = []
    for c in range(8):
        b, hf = c // 2, c % 2
        hcols = np.concatenate(
            [np.arange(D * h, D * h + D) for h in HEADS_HALF[hf]]
        )
        kcols = np.concatenate(
            [np.arange(D * g, D * g + D) for g in KV_HALF[hf]]
        )
        # Q columns: local (g-major) -> [j, g, d] blocks of 128, zero-padded
        wq_full = Wq[:, hcols]  # [896, 448], head-major h = 4g+j
        wq_pad = np.zeros((C, 512), dtype=np.float32)
        wq_pad[:, :DH] = wq_full
        wq_p = (
            wq_pad.reshape(C, 4, 4, D).transpose(0, 2, 1, 3).reshape(C, 512)
        )
        wo_full = Wo[hcols, :]  # [448, 896]
        wo_pad = np.zeros((512, C), dtype=np.float32)
        wo_pad[:DH] = wo_full
        maps.append(
            {
                "xt": _to_bf16(x[b].T),
                "wq": _to_bf16(_row_blocks(wq_p, 7)),
                "wk": _to_bf16(_row_blocks(Wk[:, kcols], 7)),
                "wv": _to_bf16(_row_blocks(Wv[:, kcols], 7)),
                "wo": _to_bf16(_row_blocks(wo_pad, 4)),
                "mask": _to_bf16(maskb),
                "sel": _to_bf16(selm),
            }
        )
    return maps


def run(x, Wq, Wk, Wv, Wo, trace=False):
    nc = _build()
    res = run_bass_kernel_spmd(
        nc, _in_maps(x, Wq, Wk, Wv, Wo), core_ids=list(range(8)), trace=trace
    )
    outs = [r["out"] for r in res.results]
    final = np.empty((4, T, C), np.float32)
    for b in range(4):
        final[b] = outs[2 * b] + outs[2 * b + 1]
    return final, res


def kernel(x, Wq, Wk, Wv, Wo):
    x = np.asarray(x, dtype=np.float32)
    out, _ = run(
        x,
        np.asarray(Wq, np.float32),
        np.asarray(Wk, np.float32),
        np.asarray(Wv, np.float32),
        np.asarray(Wo, np.float32),
    )
    return out


if __name__ == "__main__":
    pass
